# revision 22
# baseline (speedup 1.0000x reference)
"""Multi-head attention (B=2,T=2048,C=1024,H=16,RoPE,causal) on 8 TRN2 cores.

Sharding: core c -> (batch b = c//4, head-group g = c%4, heads [4g,4g+4)).
Each core computes QKV projection for its 4 heads against x[b], RoPE,
causal attention in transposed-score layout [s, t], and the output
projection rows t' in [512g, 512g+512) of y[b] (the reference's
(B,H,T,Dh)->(B,T,C) reshape makes output blocks head-disjoint).

Perf structure (v2):
- v_nat carries 64 ones-columns so the softmax denominators appear
  pre-broadcast in PSUM rows 64:128 of the AV output; normalization is
  recip + one mul (no partition_broadcast / zrow copies).
- Output projection contracts 64-row chunks of att directly via strided
  stationary APs (no scramble copies), with hl0/hl1 row-tile pairs that
  pack in the PE array.  Weights are staged with both partition halves
  duplicated so each row group streams its own copy.
- Diagonal score chunks compute/exp/accumulate only the causal column
  range.
"""
import math
import sys

sys.path.insert(0, '/opt/trn_rl_repo')
sys.path.insert(0, '/opt/pypackages')

import ml_dtypes
import numpy as np
from contextlib import ExitStack

import concourse.bass as bass  # noqa: F401
import concourse.tile as tile
from concourse import bacc, mybir
from concourse.bass_utils import run_bass_kernel_spmd

BF16 = mybir.dt.bfloat16
F32 = mybir.dt.float32
NPBF16 = ml_dtypes.bfloat16

B, T, C, H, Dh = 2, 2048, 1024, 16, 64
HALF = Dh // 2          # 32
NCORES = 8
HPC = 4                 # heads per core
CPC = HPC * Dh          # channels per core = 256
SCALE = 1.0 / math.sqrt(Dh)
TT = 512                # t-tile width
NTT = T // TT           # 4
SC = 128                # s-chunk width

_compiled_nc = None


def _build_nc(dbg=False):
    nc = bacc.Bacc("TRN2", target_bir_lowering=False, debug=False)

    # host-prearranged so every DMA is contiguous per partition (128 descr.)
    xA = nc.dram_tensor("xA", [128, NTT * 8 * TT], BF16,
                        kind="ExternalInput").ap()
    wqA = nc.dram_tensor("wqA", [128, 2 * 8 * 3 * 128], BF16,
                         kind="ExternalInput").ap()
    wp64 = nc.dram_tensor("wp64", [128, 16 * C], BF16, kind="ExternalInput").ap()
    cosx = nc.dram_tensor("cosx", [128, T], BF16, kind="ExternalInput").ap()
    sinx = nc.dram_tensor("sinx", [128, T], BF16, kind="ExternalInput").ap()
    rt = nc.dram_tensor("rt", [128, 128], BF16, kind="ExternalInput").ap()
    ident = nc.dram_tensor("ident", [128, 128], BF16, kind="ExternalInput").ap()
    mask01 = nc.dram_tensor("mask01", [128, 128], BF16, kind="ExternalInput").ap()
    yblk = nc.dram_tensor("yblk", [512, C], F32, kind="ExternalOutput").ap()
    if dbg:
        adbg = nc.dram_tensor("adbg", [128, 2, T], BF16,
                              kind="ExternalOutput").ap()
        zdbg = nc.dram_tensor("zdbg", [64, 2, T], F32,
                              kind="ExternalOutput").ap()

    with tile.TileContext(nc) as tc, ExitStack() as ctx:
        const = ctx.enter_context(tc.tile_pool(name="const", bufs=1))
        qkpool = ctx.enter_context(tc.tile_pool(name="qk", bufs=2))
        vpool = ctx.enter_context(tc.tile_pool(name="vnat", bufs=4))
        attp = ctx.enter_context(tc.tile_pool(name="attp", bufs=2))
        tmp = ctx.enter_context(tc.tile_pool(name="tmp", bufs=3))
        rzpool = ctx.enter_context(tc.tile_pool(name="rzp", bufs=2))
        psA = ctx.enter_context(tc.tile_pool(name="psA", bufs=2, space="PSUM"))
        psQK = ctx.enter_context(tc.tile_pool(name="psQK", bufs=2, space="PSUM"))
        psAT = ctx.enter_context(tc.tile_pool(name="psAT", bufs=2, space="PSUM"))

        # ---- constants, ordered so the first QKV matmul's deps land first
        xr = xA.rearrange("p (tt cc t) -> p tt cc t", tt=NTT, cc=8)
        wr = wqA.rearrange("p (hp gi cc f) -> p hp gi cc f", hp=2, gi=3, cc=8)
        wqkv_sb = const.tile([128, 2, 3, 8, 128], BF16)
        nc.sync.dma_start(wqkv_sb[:, 0, 0], wr[:, 0, 0])
        x_sb = []
        for tt in range(NTT):
            x_sb.append(const.tile([128, 8, TT], BF16, name=f"x_sb{tt}"))
        nc.sync.dma_start(x_sb[0][:], xr[:, 0])
        nc.sync.dma_start(wqkv_sb[:, 0, 1:3], wr[:, 0, 1:3])
        nc.sync.dma_start(wqkv_sb[:, 1], wr[:, 1])
        rt_sb = const.tile([128, 128], BF16)
        nc.sync.dma_start(rt_sb[:], rt[:])
        cos_sb = const.tile([128, T], BF16)
        nc.sync.dma_start(cos_sb[:], cosx[:])
        sin_sb = const.tile([128, T], BF16)
        nc.sync.dma_start(sin_sb[:], sinx[:])
        id_sb = const.tile([128, 128], BF16)
        nc.sync.dma_start(id_sb[:], ident[:])
        mask_sb = const.tile([128, 128], BF16)
        nc.sync.dma_start(mask_sb[:], mask01[:])
        for tt in range(1, NTT):
            nc.sync.dma_start(x_sb[tt][:], xr[:, tt])
        wt_sb = const.tile([128, 16, C], BF16)
        nc.sync.dma_start(wt_sb[:], wp64.rearrange("p (j o) -> p j o", o=C))

        for hp in range(2):
            q_sb = qkpool.tile([128, T], BF16, tag="q", name=f"q_{hp}")
            k_sb = qkpool.tile([128, T], BF16, tag="k", name=f"k_{hp}")
            att_sb = attp.tile([128, T], BF16, tag="att", name=f"att_{hp}")
            # v_nat[hl]: cols 0:64 = V^T chunk, cols 64:128 = ones (gives the
            # softmax denominator, replicated over 64 PSUM rows, for free)
            v_nat = [vpool.tile([128, T // SC, 128], BF16, tag="vnat",
                                name=f"vnat_{hp}_{hl}")
                     for hl in range(2)]
            for hl in range(2):
                nc.gpsimd.memset(v_nat[hl][:, :, Dh:128], 1.0)

            # ---- stage A: QKV projection + RoPE + v transpose ----
            # order per tt: q-proj, k-proj, rot(q), v-proj, rot(k), v-transp.
            # so each rot matmul's gb-copy dependency completes during the
            # preceding projection chain instead of stalling the PE.
            for tt in range(NTT):
                ts = slice(tt * TT, (tt + 1) * TT)

                def proj(gi, nm):
                    gps = psA.tile([128, TT], F32, tag="mm",
                                   name=f"gps_{hp}_{tt}_{nm}")
                    for cc in range(8):
                        nc.tensor.matmul(
                            gps[:], wqkv_sb[:, hp, gi, cc, :],
                            x_sb[tt][:, cc, :],
                            start=(cc == 0), stop=(cc == 7))
                    gb = tmp.tile([128, TT], BF16, tag="gb",
                                  name=f"gb_{hp}_{tt}_{nm}")
                    nc.any.tensor_copy(gb[:], gps[:])
                    return gb

                def rot_and_rope(gb, dest, nm):
                    rot_ps = psA.tile([128, TT], F32, tag="mm",
                                      name=f"rot_{hp}_{tt}_{nm}")
                    nc.tensor.matmul(rot_ps[:], rt_sb[:], gb[:],
                                     start=True, stop=True)
                    gc = tmp.tile([128, TT], BF16, tag="gc")
                    nc.gpsimd.tensor_mul(gc[:], gb[:], cos_sb[:, ts])
                    gs = tmp.tile([128, TT], BF16, tag="gs")
                    nc.vector.tensor_mul(gs[:], rot_ps[:], sin_sb[:, ts])
                    nc.vector.tensor_add(dest[:, ts], gc[:], gs[:])

                gb_q = proj(0, "q")
                gb_k = proj(1, "k")
                rot_and_rope(gb_q, q_sb, "q")
                gb_v = proj(2, "v")
                rot_and_rope(gb_k, k_sb, "k")
                for st in range(TT // 128):
                    ci = tt * 4 + st
                    tps = psA.tile([128, 128], BF16, tag="mm",
                                   name=f"tps_{hp}_{tt}_{st}")
                    nc.tensor.transpose(
                        tps[:], gb_v[:, st * 128:(st + 1) * 128], id_sb[:])
                    nc.any.tensor_copy(v_nat[0][:, ci, 0:Dh], tps[:, 0:64])
                    nc.any.tensor_copy(v_nat[1][:, ci, 0:Dh], tps[:, 64:128])

            # ---- stage B: causal attention (scoresT layout [s, t]) ----
            for tt in range(NTT):
                t0 = tt * TT
                outs = [psAT.tile([128, TT], F32, tag="attps",
                                  name=f"attps_{hp}_{tt}_{hl}")
                        for hl in range(2)]
                njs = 4 * tt + 4
                for j in range(njs):
                    sj = slice(j * SC, (j + 1) * SC)
                    off = (j - 4 * tt) * 128 if j >= 4 * tt else -1
                    lo = max(off, 0)          # causal col start within tile
                    qk = psQK.tile([128, 2 * TT], F32, tag="qk",
                                   name=f"qk_{hp}_{tt}_{j}")
                    for hl in range(2):
                        hb = hl * 64
                        nc.tensor.matmul(
                            qk[:, hl * TT + lo:(hl + 1) * TT],
                            k_sb[hb:hb + 64, sj],
                            q_sb[hb:hb + 64, t0 + lo:t0 + TT],
                            start=True, stop=True)
                    pb = tmp.tile([128, 2 * TT], BF16, tag="probs")
                    qk_r = qk.rearrange("p (h t) -> p h t", h=2)
                    pb_r = pb.rearrange("p (h t) -> p h t", h=2)
                    # one exp instruction over both heads' causal ranges
                    nc.scalar.activation(
                        pb_r[:, :, lo:TT], qk_r[:, :, lo:TT],
                        mybir.ActivationFunctionType.Exp, scale=SCALE)
                    if off >= 0:
                        mask_b = mask_sb.rearrange(
                            "p (o t) -> p o t", o=1).broadcast_to([128, 2, 128])
                        nc.vector.tensor_mul(pb_r[:, :, lo:lo + 128],
                                             pb_r[:, :, lo:lo + 128], mask_b)
                    for hl in range(2):
                        nc.tensor.matmul(
                            outs[hl][:, lo:TT], v_nat[hl][:, j, :],
                            pb[:, hl * TT + lo:(hl + 1) * TT],
                            start=(j == 0), stop=(j == njs - 1))
                # normalize: denominators sit pre-broadcast in rows 64:128
                for hl in range(2):
                    op = outs[hl]
                    # custom-DVE recip ignores partition offsets, so first
                    # move the denominators to base partition 0 with a
                    # standard copy (supports shifted bases), then recip+mul
                    # entirely at base 0.
                    ztmp = rzpool.tile([64, TT], F32, tag="zt",
                                       name=f"zt_{hp}_{tt}_{hl}")
                    nc.any.tensor_copy(ztmp[:], op[64:128, :])
                    rz = rzpool.tile([64, TT], F32, tag="rz",
                                     name=f"rz_{hp}_{tt}_{hl}")
                    nc.vector.reciprocal_approx_fast(out=rz[:], in_=ztmp[:])
                    nc.vector.tensor_mul(
                        att_sb[hl * 64:hl * 64 + 64, t0:t0 + TT],
                        op[0:Dh, :], rz[:])
                    if dbg:
                        nc.sync.dma_start(zdbg[:, hl, t0:t0 + TT], rz[:])

            # ---- output projection: contract 64-row chunks of att ----
            # y row t' = r0 + kk draws from head h = 2hp+hl positions
            # t = 16kk + j, channel c' = 64j + d:
            #   Y[kk, o] = sum_j sum_d att[d, 16kk+j] * wpT[64j+d, o]
            # lhsT chunk j = att[hl*64:+64, j::16]  (strided stationary);
            # rhs = wt_sb (both partition halves hold the same rows, so the
            # hl0/hl1 row-tile pairs each stream their own copy and pack).
            if dbg:
                nc.sync.dma_start(adbg[:, hp, :], att_sb[:])
            att_v = att_sb.rearrange("p (kk j) -> p kk j", j=16)
            ypss = [psQK.tile([128, 2 * TT], F32, tag="qk",
                              name=f"yps_{hp}_{hl}") for hl in range(2)]
            for j in range(16):
                for ot in range(2):
                    for hl in range(2):
                        nc.tensor.matmul(
                            ypss[hl][:, ot * TT:(ot + 1) * TT],
                            att_v[hl * 64:hl * 64 + 64, :, j],
                            wt_sb[hl * 64:hl * 64 + 64, j,
                                  ot * TT:(ot + 1) * TT],
                            start=(j == 0), stop=(j == 15))
            for hl in range(2):
                r0 = (hp * 2 + hl) * 128
                for ot in range(2):
                    yo = tmp.tile([128, TT], F32, tag="yo")
                    if hl == 0:
                        nc.scalar.copy(yo[:], ypss[hl][:, ot * TT:(ot + 1) * TT])
                    else:
                        nc.vector.tensor_copy(yo[:],
                                              ypss[hl][:, ot * TT:(ot + 1) * TT])
                    nc.sync.dma_start(
                        yblk[r0:r0 + 128, ot * TT:(ot + 1) * TT], yo[:])

    nc.compile()
    return nc


def _get_nc():
    global _compiled_nc
    if _compiled_nc is None:
        _compiled_nc = _build_nc()
    return _compiled_nc


def _host_tables():
    pos = np.arange(T, dtype=np.float32)[:, None]
    inv = np.exp(np.arange(0, Dh, 2, dtype=np.float32)
                 * (-math.log(10000.0) / Dh))
    ang = pos * inv                       # (T, 32)
    sin, cos = np.sin(ang), np.cos(ang)   # (T, 32)
    idx = np.arange(128) % HALF           # d % 32
    cos_ext = cos[:, idx].T.astype(NPBF16)  # (128, T)
    sin_ext = sin[:, idx].T.astype(NPBF16)

    R = np.zeros((128, 128), dtype=np.float32)
    for blk in (0, 64):
        for m in range(HALF):
            R[blk + m, blk + m + HALF] = -1.0
            R[blk + m + HALF, blk + m] = 1.0
    rt = np.ascontiguousarray(R.T).astype(NPBF16)

    s_i = np.arange(128)[:, None]
    t_i = np.arange(128)[None, :]
    mask01 = (t_i >= s_i).astype(np.float32).astype(NPBF16)
    ident = np.eye(128, dtype=np.float32).astype(NPBF16)
    return cos_ext, sin_ext, rt, mask01, ident


def kernel(x, w_qkv, w_proj):
    x = np.asarray(x)
    w_qkv = np.asarray(w_qkv)
    w_proj = np.asarray(w_proj)
    nc = _get_nc()
    in_maps = build_in_maps(x, w_qkv, w_proj)
    res = run_bass_kernel_spmd(nc, in_maps, core_ids=list(range(NCORES)))
    y = np.zeros((B, T, C), dtype=np.float32)
    for c in range(NCORES):
        b, g = c // 4, c % 4
        y[b, 512 * g:512 * g + 512, :] = res.results[c]["yblk"]
    return y


def build_in_maps(x, w_qkv, w_proj):
    cos_ext, sin_ext, rt, mask01, ident = _host_tables()
    wq4 = w_qkv.reshape(3, H, Dh, C)
    # wt64[p, j, o] = wpT[64j + (p % 64), o], both partition halves equal
    wpT = w_proj.T.astype(np.float32)                      # (C, C)
    half64 = wpT.reshape(16, 64, C).transpose(1, 0, 2)     # (64, 16, C)
    wt64 = np.concatenate([half64, half64], axis=0)        # (128, 16, C)
    wp64 = np.ascontiguousarray(
        wt64.reshape(128, 16 * C)).astype(NPBF16)
    in_maps = []
    for c in range(NCORES):
        b, g = c // 4, c % 4
        hs = slice(4 * g, 4 * g + 4)
        wq = wq4[0, hs].reshape(CPC, C)
        wk = wq4[1, hs].reshape(CPC, C)
        wv = wq4[2, hs].reshape(CPC, C)
        wqkvT = np.concatenate([wq, wk, wv], axis=0).T    # (C, 768)
        # wqA[p, hp, gi, cc, f] = wqkvT[cc*128+p, gi*256 + hp*128 + f]
        wqA = (wqkvT.reshape(8, 128, 3, 2, 128)
               .transpose(1, 3, 2, 0, 4).reshape(128, 6144))
        xT = x[b].T                                        # (C, T)
        # xA[p, tt, cc, t] = xT[cc*128+p, tt*512+t]
        xAr = (xT.reshape(8, 128, NTT, TT)
               .transpose(1, 2, 0, 3).reshape(128, NTT * 8 * TT))
        in_maps.append({
            "xA": np.ascontiguousarray(xAr).astype(NPBF16),
            "wqA": np.ascontiguousarray(wqA).astype(NPBF16),
            "wp64": wp64,
            "cosx": cos_ext, "sinx": sin_ext,
            "rt": rt, "ident": ident, "mask01": mask01,
        })
    return in_maps
